# revision 1
# baseline (speedup 1.0000x reference)
"""Trainium2 Bass kernel for the KPC fusion module (dense_transformer).

Sequence-parallel over N (8 cores x 4096 points x 2 batches). Per core the
fused double-softmax attention runs as a software-pipelined stream over 17
l-tiles x 16 point-chunks:

    S^T = K2^T @ x            (PE, f32r; q folded into K2)
    P   = exp(S^T)            (odd tiles: ACT exact Exp;
                               even tiles: DVE Schraudolph fast-exp in the
                               bf16 domain, one tensor_scalar into int16)
    [Num;Den] += v^T @ P      (PE PSUM accumulation, f32r / bf16)

Emission interleaves scores LA tiles ahead of the accumulates so the PE
(the bottleneck engine) never waits on exp; exp work is split between ACT
and DVE. All constant matmuls use f32r (f32 matmuls cost 4 cycles/row).

Epilogue: LayerNorm is scale-invariant per point, so instead of dividing by
softmax denominators we multiply each feature group by the *other* group's
denominator (no big reciprocal). BN batch stats: one 48-byte AllReduce,
preceded by a dummy warm-up AllReduce at program start (halves its latency).
"""

import numpy as np

B = 2
C = 6
N = 32768
MID = 3
NCV = 3
L = 2048
NCORES = 8
NS = N // NCORES
EPS = 1e-5
LT = 17                   # 16 intra l-tiles of 128 + 1 inter tile
NCH = NS // 512           # 8 point chunks of 512 per batch
SCH = 8
LP = 4
LEXT = LT * 128

LA = 3                    # accum matmuls trail score matmuls by LA tiles
# bf16-domain Schraudolph fast-exp: exp(x) ~= bitcast_bf16(int16(A*x + B))
A_EXP = float(2 ** 7 / np.log(2))
B_EXP = float(127 * 2 ** 7 - 366393.0 / 65536.0)

_cache = {}


def _host_consts(inputs):
    """Weight algebra + constant routing matrices, packed into one block."""
    f32 = np.float32
    Wa, Wav, Wb, Wbv, Wc, Wd = (np.asarray(inputs[k], f32) for k in
                                ["Wa", "Wav", "Wb", "Wbv", "Wc", "Wd"])
    Watt = np.asarray(inputs["Watt"], f32)
    ln_g, ln_b = np.asarray(inputs["ln_gamma"], f32), np.asarray(inputs["ln_beta"], f32)
    Wpl, bpl = np.asarray(inputs["Wpl"], f32), np.asarray(inputs["bpl"], f32)
    Wpn, bpn = np.asarray(inputs["Wpn"], f32), np.asarray(inputs["bpn"], f32)

    scale = np.sqrt(f32(MID))
    Wc_s = (Wc / scale).astype(f32)
    WA = (Wc_s.T @ Wb).astype(f32)            # [6,6] K2_intra = WA @ ga
    WB = (Wc_s.T @ Wa).astype(f32)            # [6,6] K2_inter = WB @ gi
    wbar = Wpn.mean(axis=0).astype(f32)
    bbar = float(bpn.mean())
    W1 = (Wd * ln_g[None, :]).astype(f32)
    c0 = (Wd @ ln_b).astype(f32)

    def bd(m, k=2):
        """k-fold block-diagonal stack of m."""
        r, c = m.shape
        out = np.zeros((k * r, k * c), f32)
        for i in range(k):
            out[i * r:(i + 1) * r, i * c:(i + 1) * c] = m
        return out

    consts = {}
    Watt_map = np.zeros((72, 12), f32)
    for s in range(12):
        for c in range(C):
            Watt_map[s * 6 + c, s] = Watt[c]
    consts["Watt_map"] = Watt_map

    map_l = np.zeros((12, 3), f32)
    map_l2 = np.zeros((3, 12), f32)
    map_n = np.zeros((12, 4), f32)
    map_n2 = np.zeros((4, 12), f32)
    for s in range(12):
        n, lp = divmod(s, LP)
        map_l[s, n] = 1.0
        map_l2[n, s] = 1.0
        map_n[s, lp] = 1.0
        map_n2[lp, s] = 1.0
    consts["map_l"] = map_l
    consts["map_l2"] = map_l2
    consts["map_n"] = map_n
    consts["map_n2"] = map_n2

    rep_c = np.zeros((12, 72), f32)
    for s in range(12):
        for c in range(C):
            rep_c[s, s * 6 + c] = 1.0
    consts["rep_c"] = rep_c

    map_red_ci = np.zeros((72, 18), f32)
    map_pl = np.zeros((72, 18), f32)
    map_ci = np.zeros((72, 24), f32)
    map_pn = np.zeros((72, 24), f32)
    for s in range(12):
        n, lp = divmod(s, LP)
        for c in range(C):
            p = s * 6 + c
            map_red_ci[p, c * 3 + n] = 1.0
            for m in range(MID):
                map_pl[p, c * 3 + m] = Wpl[m, n] / L
            map_ci[p, c * LP + lp] = 1.0
            map_pn[p, c * LP + lp] = wbar[n]
    consts["map_red_ci"] = map_red_ci
    consts["map_pl"] = map_pl
    consts["map_ci"] = map_ci
    consts["map_pn"] = map_pn

    consts["bpl_rep"] = np.tile(bpl, C).reshape(18, 1)

    WA_map = np.zeros((24, 24), f32)
    for lp in range(LP):
        for c in range(C):
            for c2 in range(C):
                WA_map[c * LP + lp, c2 * LP + lp] = WA[c2, c]
    consts["WA_map"] = WA_map
    consts["WB_T"] = WB.T.copy()
    consts["WavT"] = Wav.T.copy()
    consts["WbvT"] = Wbv.T.copy()

    # epilogue maps on per-batch (c, s) packed [48] rows
    map_mean_h = np.zeros((48, 8), f32)
    rep16_h = np.zeros((8, 48), f32)
    W1_map_h = np.zeros((48, 48), f32)
    c0_rep_h = np.zeros((48, 1), f32)
    bn_map_h = np.zeros((48, 6), f32)
    rep_ad_h = np.zeros((6, 48), f32)
    for c in range(C):
        for s in range(SCH):
            p = c * SCH + s
            map_mean_h[p, s] = 1.0 / C
            rep16_h[s, p] = 1.0
            for o in range(C):
                W1_map_h[p, o * SCH + s] = W1[o, c]
            c0_rep_h[p, 0] = c0[c]
            bn_map_h[p, c] = 1.0
    for c in range(C):
        for s in range(SCH):
            rep_ad_h[c, c * SCH + s] = 1.0
    consts["map_mean_h"] = map_mean_h
    consts["rep16_h"] = rep16_h
    consts["W1_map_h"] = W1_map_h
    consts["c0_rep_h"] = c0_rep_h
    consts["bn_map_h"] = bn_map_h
    consts["rep_ad_h"] = rep_ad_h

    consts["bn_gamma_c"] = np.asarray(inputs["bn_gamma"], f32).reshape(6, 1)
    consts["bn_beta_c"] = np.asarray(inputs["bn_beta"], f32).reshape(6, 1)

    # pack everything into one [128, F] block (one DMA on device)
    layout = {}
    off = 0
    for k, v in consts.items():
        r, ccols = v.shape
        layout[k] = (r, off, ccols)
        off += ccols
    pack = np.zeros((128, off), f32)
    for k, v in consts.items():
        r, o, ccols = layout[k]
        pack[:r, o:o + ccols] = v
    return pack, layout, bbar


def _build(pack_shape, layout, bbar):
    import concourse.bacc as bacc
    import concourse.mybir as mybir
    import concourse.tile as tile

    dt = mybir.dt
    f32 = dt.float32
    f32r = dt.float32r
    i16 = dt.int16
    bf16 = dt.bfloat16
    AF = mybir.ActivationFunctionType
    ALU = mybir.AluOpType
    AX = mybir.AxisListType

    nc = bacc.Bacc(
        "TRN2", target_bir_lowering=False, debug=False, num_devices=NCORES
    )

    x_d = nc.dram_tensor("x_sh", [B, C, NS], f32, kind="ExternalInput")
    xbf_d = nc.dram_tensor("x_bf", [B, C, NS], bf16, kind="ExternalInput")
    curves_d = nc.dram_tensor("curves", [B, C, NCV, L], f32, kind="ExternalInput")
    pack_d = nc.dram_tensor("cpack", list(pack_shape), f32, kind="ExternalInput")
    out_d = nc.dram_tensor("out", [B, C, NS], f32, kind="ExternalOutput")

    with tile.TileContext(nc) as tc:
        with (
            tc.tile_pool(name="const", bufs=1) as constp,
            tc.tile_pool(name="pre", bufs=1) as pre,
            tc.tile_pool(name="aux", bufs=2, space="PSUM") as aux,
            tc.tile_pool(name="spsum", bufs=4, space="PSUM") as spool,
            tc.tile_pool(name="ndpsum", bufs=2, space="PSUM") as ndpool,
            tc.tile_pool(name="pact", bufs=6) as pact,
            tc.tile_pool(name="pdve", bufs=6) as pdve,
            tc.tile_pool(name="epi", bufs=1) as epi,
            tc.tile_pool(name="dram", bufs=1, space="DRAM") as dram,
            nc.allow_low_precision(reason="f32r outputs carry full f32 bits"),
        ):
            # ---- input loads (preprocessing feeds first) ----
            pk = constp.tile(list(pack_shape), f32r, tag="cpack")
            nc.sync.dma_start(pk[:], pack_d[:].bitcast(f32r))
            cs = {k: pk[0:r, o:o + w] for k, (r, o, w) in layout.items()}
            cpk_all = []
            for b in range(B):
                cpk = pre.tile([72, 512], f32r, tag=f"cpk{b}")
                nc.sync.dma_start(
                    cpk[:],
                    curves_d[b].rearrange("c n (lp j) -> c (n lp) j", j=512)
                    .transpose([1, 0, 2]).bitcast(f32r),
                )
                cpk_all.append(cpk)
            x_sb = []
            for b in range(B):
                t = pre.tile([C, NS], bf16, tag=f"x{b}")
                nc.sync.dma_start(t[:], xbf_d[b])
                x_sb.append(t)
            xp_sb = []
            for b in range(B):
                t = epi.tile([48, 512], f32, name=f"xp{b}", tag=f"xp{b}")
                nc.sync.dma_start(
                    t[:], x_d[b].rearrange("c (s j) -> (c s) j", j=512)
                )
                xp_sb.append(t)

            # ---- CC warm-up: dummy AllReduce well before the real one ----
            cc_in = dram.tile([6, 2], f32, tag="ccin")
            cc_out = dram.tile([6, 2], f32, tag="ccout")
            warm = epi.tile([6, 2], f32, tag="warm")
            nc.vector.memset(warm[:], 0.0)
            nc.gpsimd.dma_start(cc_in[:], warm[:])
            nc.gpsimd.collective_compute(
                "AllReduce", mybir.AluOpType.add,
                replica_groups=[list(range(NCORES))],
                ins=[cc_in[:].opt()], outs=[cc_out[:].opt()],
            )

            # ---- preprocessing (per batch, all-f32r matmuls) ----
            E_att, sm_l, sm_n = [None, None], [None, None], [None, None]
            prod_i, prod_n, red_i = [None, None], [None, None], [None, None]
            gi_c, ga_sb, ga_c6 = [None, None], [None, None], [None, None]
            K2ext, vext_bf = [None, None], [None, None]
            red_p0 = None

            for b in range(B):
                att_ps = aux.tile([12, 512], f32, tag="ps")
                nc.tensor.matmul(att_ps[:], cs["Watt_map"], cpk_all[b][:],
                                 start=True, stop=True)
                E_att[b] = pre.tile([12, 512], f32r, name=f"eatt{b}",
                                    tag=f"eatt{b}")
                nc.scalar.activation(E_att[b][:], att_ps[:], AF.Exp)
            for b in range(B):
                sums_s = pre.tile([12, 1], f32, name=f"sums{b}",
                                  tag=f"sums{b}")
                nc.vector.reduce_sum(sums_s[:],
                                     E_att[b][:].bitcast(f32), axis=AX.X)
                dl_ps = aux.tile([3, 1], f32, tag="ps")
                nc.tensor.matmul(dl_ps[:], cs["map_l"].bitcast(f32),
                                 sums_s[:], start=True, stop=True)
                rl = pre.tile([3, 1], f32, name=f"rl{b}", tag=f"rl{b}")
                nc.vector.reciprocal(rl[:], dl_ps[:])
                rl_rep_ps = aux.tile([12, 1], f32, tag="ps")
                nc.tensor.matmul(rl_rep_ps[:], cs["map_l2"].bitcast(f32),
                                 rl[:], start=True, stop=True)
                rl_rep = pre.tile([12, 1], f32, name=f"rlrep{b}",
                                  tag=f"rlrep{b}")
                nc.vector.tensor_copy(rl_rep[:], rl_rep_ps[:])
                sm_l[b] = pre.tile([12, 512], f32r, name=f"sml{b}",
                                   tag=f"sml{b}")
                nc.vector.tensor_scalar_mul(sm_l[b][:],
                                            E_att[b][:].bitcast(f32),
                                            rl_rep[:])
            for b in range(B):
                dn_ps = aux.tile([4, 512], f32, tag="ps")
                nc.tensor.matmul(dn_ps[:], cs["map_n"], E_att[b][:],
                                 start=True, stop=True)
                rn_f = pre.tile([4, 512], f32, name=f"rnf{b}",
                                tag=f"rnf{b}")
                nc.vector.reciprocal_approx_fast(rn_f[:], dn_ps[:])
                rn = pre.tile([4, 512], f32r, name=f"rn{b}", tag=f"rn{b}")
                nc.vector.tensor_copy(rn[:], rn_f[:])
                rn_rep_ps = aux.tile([12, 512], f32, tag="ps")
                nc.tensor.matmul(rn_rep_ps[:], cs["map_n2"], rn[:],
                                 start=True, stop=True)
                sm_n[b] = pre.tile([12, 512], f32r, name=f"smn{b}",
                                   tag=f"smn{b}")
                nc.vector.tensor_tensor(sm_n[b][:],
                                        E_att[b][:].bitcast(f32),
                                        rn_rep_ps[:], ALU.mult)
            for b in range(B):
                sml_rep_ps = aux.tile([72, 512], f32, tag="ps")
                nc.tensor.matmul(sml_rep_ps[:], cs["rep_c"], sm_l[b][:],
                                 start=True, stop=True)
                prod_i[b] = pre.tile([72, 512], f32, name=f"prodi{b}",
                                     tag=f"prodi{b}")
                nc.vector.tensor_tensor(prod_i[b][:],
                                        cpk_all[b][:].bitcast(f32),
                                        sml_rep_ps[:], ALU.mult)
                smn_rep_ps = aux.tile([72, 512], f32, tag="ps")
                nc.tensor.matmul(smn_rep_ps[:], cs["rep_c"], sm_n[b][:],
                                 start=True, stop=True)
                prod_n[b] = pre.tile([72, 512], f32r, name=f"prodn{b}",
                                     tag=f"prodn{b}")
                nc.vector.tensor_tensor(prod_n[b][:],
                                        cpk_all[b][:].bitcast(f32),
                                        smn_rep_ps[:], ALU.mult)
            red_p0 = pre.tile([72, 1], f32, tag="redp0")
            nc.vector.reduce_sum(red_p0[:],
                                 cpk_all[0][:].bitcast(f32), axis=AX.X)
            for b in range(B):
                red_i[b] = pre.tile([72, 1], f32, name=f"redi{b}",
                                    tag=f"redi{b}")
                nc.vector.reduce_sum(red_i[b][:], prod_i[b][:],
                                     axis=AX.X)
            for b in range(B):
                gi_ps = aux.tile([18, 1], f32, tag="ps")
                nc.tensor.matmul(gi_ps[:], cs["map_red_ci"].bitcast(f32),
                                 red_i[b][:], start=True, stop=False)
                nc.tensor.matmul(gi_ps[:], cs["map_pl"].bitcast(f32),
                                 red_p0[:], start=False, stop=True)
                gi18 = pre.tile([18, 1], f32, name=f"gi18{b}",
                                tag=f"gi18{b}")
                nc.vector.tensor_scalar_add(gi18[:], gi_ps[:],
                                            cs["bpl_rep"].bitcast(f32))
                gi_c[b] = pre.tile([C, 3], f32, name=f"gic{b}",
                                   tag=f"gic{b}")
                nc.sync.dma_start(gi_c[b][:], gi18[:])
            for b in range(B):
                ga_ps = aux.tile([24, 512], f32, tag="ps")
                nc.tensor.matmul(ga_ps[:], cs["map_ci"], prod_n[b][:],
                                 start=True, stop=False)
                nc.tensor.matmul(ga_ps[:], cs["map_pn"], cpk_all[0][:],
                                 start=False, stop=True)
                ga_sb[b] = pre.tile([24, 512], f32r, name=f"ga{b}",
                                    tag=f"ga{b}")
                nc.scalar.activation(ga_sb[b][:], ga_ps[:], AF.Copy,
                                     bias=float(bbar))
                ga_c6[b] = pre.tile([C, L], f32, name=f"gac6{b}",
                                    tag=f"gac6{b}")
                nc.sync.dma_start(ga_c6[b][:], ga_sb[b][:].bitcast(f32))
            for b in range(B):
                k2_ps = aux.tile([24, 512], f32, tag="ps")
                nc.tensor.matmul(k2_ps[:], cs["WA_map"], ga_sb[b][:],
                                 start=True, stop=True)
                k2_24 = pre.tile([24, 512], bf16, name=f"k224{b}",
                                 tag=f"k224{b}")
                nc.vector.tensor_copy(k2_24[:], k2_ps[:])
                k2e = pre.tile([C, LEXT], bf16, name=f"k2e{b}", tag=f"k2e{b}")
                nc.vector.memset(k2e[:, 2048:], 0.0)
                nc.sync.dma_start(k2e[:, 0:2048], k2_24[:])
                k2i_ps = aux.tile([C, 3], f32, tag="ps")
                nc.tensor.matmul(k2i_ps[:], cs["WB_T"].bitcast(f32),
                                 gi_c[b][:], start=True, stop=True)
                nc.vector.tensor_copy(k2e[:, 2048:2051], k2i_ps[:])
                K2ext[b] = k2e
            for b in range(B):
                vps = aux.tile([128, LT, 8], f32, tag="ps")
                nc.vector.memset(vps[:], 0.0)
                for t in range(16):
                    nc.tensor.matmul(
                        vps[:, t, 0:3], ga_c6[b][:, 128 * t: 128 * (t + 1)],
                        cs["WbvT"].bitcast(f32), start=True, stop=True,
                    )
                nc.tensor.matmul(vps[0:3, 16, 4:7], gi_c[b][:],
                                 cs["WavT"].bitcast(f32),
                                 start=True, stop=True)
                nc.vector.memset(vps[:, 0:16, 3:4], 1.0)
                nc.vector.memset(vps[0:3, 16, 7:8], 1.0)
                veb = pre.tile([128, LT, 8], bf16, name=f"veb{b}",
                               tag=f"veb{b}")
                nc.vector.tensor_copy(veb[:], vps[:])
                vext_bf[b] = veb

            # ---- main loop: software-pipelined fused double attention ----
            cf_p, mult_p, nd_sb = [], [], []
            for b in range(B):
                cf_p.append(epi.tile([48, 512], f32, name=f"cfp{b}",
                                     tag=f"cfp{b}"))
                mult_p.append(epi.tile([48, 512], f32, name=f"multp{b}",
                                       tag=f"multp{b}"))
                nd_sb.append(epi.tile([8, NCH, 512], f32, name=f"nd{b}",
                                      tag=f"nd{b}"))
            eps16 = epi.tile([16, 1], f32, tag="eps16")
            nc.vector.memset(eps16[:], EPS)
            y_b = [None, None]
            partials_b = [None, None]

            def epi_local(b):
                """Per-batch epilogue generator ([48,512] c,s rows)."""
                cfn = epi.tile([48, 512], f32r, name=f"cfn{b}", tag=f"cfn{b}")
                nc.vector.tensor_tensor(cfn[:], cf_p[b][:], mult_p[b][:],
                                        ALU.mult)
                cf2 = epi.tile([48, 512], f32r, name=f"cf2{b}", tag=f"cf2{b}")
                nc.vector.tensor_tensor(cf2[:], cfn[:].bitcast(f32),
                                        cfn[:].bitcast(f32), ALU.mult)
                yield
                mu_ps = aux.tile([8, 512], f32, tag="ps")
                nc.tensor.matmul(mu_ps[:], cs["map_mean_h"], cfn[:],
                                 start=True, stop=True)
                mu_sb = epi.tile([8, 512], f32, name=f"mu{b}", tag=f"mu{b}")
                nc.vector.tensor_copy(mu_sb[:], mu_ps[:])
                m2_ps = aux.tile([8, 512], f32, tag="ps")
                nc.tensor.matmul(m2_ps[:], cs["map_mean_h"], cf2[:],
                                 start=True, stop=True)
                yield
                musq = epi.tile([8, 512], f32, name=f"musq{b}",
                                tag=f"musq{b}")
                nc.vector.tensor_tensor(musq[:], mu_sb[:], mu_sb[:], ALU.mult)
                var_sb = epi.tile([8, 512], f32, name=f"var{b}",
                                  tag=f"var{b}")
                nc.vector.tensor_tensor(var_sb[:], m2_ps[:], musq[:],
                                        ALU.subtract)
                yield
                sv = epi.tile([8, 512], f32, name=f"sv{b}", tag=f"sv{b}")
                nc.scalar.activation(sv[:], var_sb[:], AF.Sqrt,
                                     bias=eps16[0:8, :])
                yield
                r_f = epi.tile([8, 512], f32, name=f"rf{b}",
                               tag=f"rf{b}")
                nc.vector.reciprocal_approx_fast(r_f[:], sv[:])
                r_sb = epi.tile([8, 512], f32r, name=f"rsb{b}",
                                tag=f"rsb{b}")
                nc.vector.tensor_copy(r_sb[:], r_f[:])
                mur = epi.tile([8, 512], f32r, name=f"mur{b}",
                               tag=f"mur{b}")
                nc.vector.tensor_tensor(mur[:], mu_sb[:], r_f[:], ALU.mult)
                yield
                rrep_ps = aux.tile([48, 512], f32, tag="ps")
                nc.tensor.matmul(rrep_ps[:], cs["rep16_h"], r_sb[:],
                                 start=True, stop=True)
                murrep_ps = aux.tile([48, 512], f32, tag="ps")
                nc.tensor.matmul(murrep_ps[:], cs["rep16_h"], mur[:],
                                 start=True, stop=True)
                yield
                z1 = epi.tile([48, 512], f32, name=f"z1{b}", tag=f"z1{b}")
                nc.vector.tensor_tensor(z1[:], cfn[:].bitcast(f32),
                                        rrep_ps[:], ALU.mult)
                z = epi.tile([48, 512], f32r, name=f"z{b}", tag=f"z{b}")
                nc.vector.tensor_tensor(z[:], z1[:], murrep_ps[:],
                                        ALU.subtract)
                yield
                y_ps = aux.tile([48, 512], f32, tag="ps")
                nc.tensor.matmul(y_ps[:], cs["W1_map_h"], z[:],
                                 start=True, stop=True)
                yb = epi.tile([48, 512], f32r, name=f"yb{b}", tag=f"yb{b}")
                nc.vector.tensor_scalar_add(yb[:], y_ps[:],
                                            cs["c0_rep_h"].bitcast(f32))
                y_b[b] = yb
                yield
                y2 = epi.tile([48, 512], f32r, name=f"y2{b}", tag=f"y2{b}")
                nc.vector.tensor_tensor(y2[:], yb[:].bitcast(f32),
                                        yb[:].bitcast(f32), ALU.mult)
                bsum_ps = aux.tile([6, 512], f32, tag="ps")
                nc.tensor.matmul(bsum_ps[:], cs["bn_map_h"], yb[:],
                                 start=True, stop=True)
                bsq_ps = aux.tile([6, 512], f32, tag="ps")
                nc.tensor.matmul(bsq_ps[:], cs["bn_map_h"], y2[:],
                                 start=True, stop=True)
                yield
                pb = epi.tile([6, 2], f32, name=f"part{b}", tag=f"part{b}")
                nc.vector.reduce_sum(pb[:, 0:1], bsum_ps[:], axis=AX.X)
                nc.vector.reduce_sum(pb[:, 1:2], bsq_ps[:], axis=AX.X)
                partials_b[b] = pb

            total = B * NCH * LT
            s_tiles = [None] * 16
            p_tiles = [None] * 16
            nd_ps = None

            def chunk_of(k):
                return divmod(k // LT, NCH)  # -> (b, ch)

            def emit_score(k):
                b, ch = chunk_of(k)
                t = k % LT
                s_ps = spool.tile([128, 512], f32)
                nc.tensor.matmul(
                    s_ps[:], K2ext[b][:, 128 * t: 128 * (t + 1)],
                    x_sb[b][:, 512 * ch: 512 * (ch + 1)],
                    start=True, stop=True,
                )
                s_tiles[k % 16] = s_ps

            def emit_exp(k):
                t = k % LT
                s_ps = s_tiles[k % 16]
                if t % 2 == 1:
                    p_t = pact.tile([128, 512], bf16)
                    nc.scalar.activation(p_t[:], s_ps[:], AF.Exp)
                else:
                    p_t = pdve.tile([128, 512], i16)
                    nc.vector.tensor_scalar(p_t[:], s_ps[:], A_EXP, B_EXP,
                                            ALU.mult, ALU.add)
                p_tiles[k % 16] = p_t

            def emit_accum(k):
                nonlocal nd_ps
                b, ch = chunk_of(k)
                t = k % LT
                if t == 0:
                    nd_ps = ndpool.tile([8, 512], f32)
                p_t = p_tiles[k % 16]
                rhs = p_t[:] if t % 2 == 1 else p_t[:].bitcast(bf16)
                nc.tensor.matmul(
                    nd_ps[:], vext_bf[b][:, t, :], rhs,
                    start=(t == 0), stop=(t == LT - 1),
                )
                if t == LT - 1:
                    # copy out + feed the per-batch epilogue repack (DMAs
                    # hide under the main loop)
                    nd = nd_sb[b]
                    nc.vector.tensor_copy(nd[:, ch, :], nd_ps[:])
                    nc.sync.dma_start(cf_p[b][ch:ch + 17:8, :],
                                      nd[4:7, ch, :])
                    nc.sync.dma_start(cf_p[b][ch + 24:ch + 41:8, :],
                                      nd[0:3, ch, :])
                    for c in range(3):
                        nc.sync.dma_start(
                            mult_p[b][ch + 8 * c:ch + 8 * c + 1, :],
                            nd[3:4, ch, :])
                        nc.sync.dma_start(
                            mult_p[b][ch + 24 + 8 * c:ch + 25 + 8 * c, :],
                            nd[7:8, ch, :])

            # Pairs of scores then pairs of accums: during a score's
            # stream the next weight load is a small 6-partition K2 slice,
            # so the stream runs at full rate; the expensive 128-partition
            # vext loads only overlap other accums.
            epi0 = None
            for k in range(total):
                emit_score(k)
                emit_exp(k)
                if k % 4 == 3 and k >= 11:
                    for ka in range(k - 11, k - 7):
                        emit_accum(ka)
                    if k - 11 <= NCH * LT - 1 <= k - 8:
                        epi0 = epi_local(0)
                # interleave batch-0 epilogue under batch-1's main loop
                if k % LT == LT - 1 and epi0 is not None:
                    for _ in range(2):
                        next(epi0, None)
            for k in range(total - 8, total):
                emit_accum(k)
            if epi0 is not None:
                for _ in epi0:
                    pass
            for _ in epi_local(1):
                pass

            # ---- AllReduce of the 12 BN partial sums ----
            partials = epi.tile([6, 2], f32, tag="partials")
            nc.vector.tensor_tensor(partials[:], partials_b[0][:],
                                    partials_b[1][:], ALU.add)
            nc.gpsimd.dma_start(cc_in[:], partials[:])
            nc.gpsimd.collective_compute(
                "AllReduce", mybir.AluOpType.add,
                replica_groups=[list(range(NCORES))],
                ins=[cc_in[:].opt()], outs=[cc_out[:].opt()],
            )
            stats_g = epi.tile([6, 2], f32, tag="statsg")
            nc.gpsimd.dma_start(stats_g[:], cc_out[:])

            # ---- final BN scalars ----
            inv_cnt = 1.0 / float(B * N)
            bm2 = epi.tile([6, 2], f32, tag="bm2")
            nc.vector.tensor_scalar_mul(bm2[:], stats_g[:], inv_cnt)
            bmsq = epi.tile([6, 1], f32, tag="bmsq")
            nc.vector.tensor_tensor(bmsq[:], bm2[:, 0:1], bm2[:, 0:1],
                                    ALU.mult)
            bv = epi.tile([6, 1], f32, tag="bv")
            nc.vector.tensor_tensor(bv[:], bm2[:, 1:2], bmsq[:], ALU.subtract)
            svb = epi.tile([6, 1], f32, tag="svb")
            nc.scalar.activation(svb[:], bv[:], AF.Sqrt, bias=eps16[0:6, :])
            rb = epi.tile([6, 1], f32, tag="rb")
            nc.vector.reciprocal(rb[:], svb[:])
            a6 = epi.tile([6, 1], f32, tag="a6")
            nc.vector.tensor_tensor(a6[:], rb[:],
                                    cs["bn_gamma_c"].bitcast(f32), ALU.mult)
            t1 = epi.tile([6, 1], f32, tag="t1")
            nc.vector.tensor_tensor(t1[:], a6[:], bm2[:, 0:1], ALU.mult)
            d6 = epi.tile([6, 1], f32, tag="d6")
            nc.vector.tensor_tensor(d6[:],
                                    cs["bn_beta_c"].bitcast(f32), t1[:],
                                    ALU.subtract)

            ad_ps = aux.tile([48, 2], f32, tag="ps")
            nc.tensor.matmul(ad_ps[:, 0:1], cs["rep_ad_h"].bitcast(f32),
                             a6[:], start=True, stop=True)
            nc.tensor.matmul(ad_ps[:, 1:2], cs["rep_ad_h"].bitcast(f32),
                             d6[:], start=True, stop=True)
            ad_sb = epi.tile([48, 2], f32, tag="adsb")
            nc.vector.tensor_copy(ad_sb[:], ad_ps[:])

            # ---- BN apply + residual + LeakyReLU + store ----
            for b in range(B):
                t5 = epi.tile([48, 512], f32, name=f"t5{b}", tag=f"t5{b}")
                nc.vector.tensor_scalar(t5[:], y_b[b][:].bitcast(f32),
                                        ad_sb[:, 0:1], ad_sb[:, 1:2],
                                        ALU.mult, ALU.add)
                t6 = epi.tile([48, 512], f32, name=f"t6{b}", tag=f"t6{b}")
                nc.vector.tensor_tensor(t6[:], t5[:], xp_sb[b][:], ALU.add)
                ob = epi.tile([48, 512], f32, name=f"outp{b}",
                              tag=f"outp{b}")
                nc.vector.scalar_tensor_tensor(ob[:], t6[:], 0.2, t6[:],
                                               ALU.mult, ALU.max)
                nc.sync.dma_start(
                    out_d[b].rearrange("c (s j) -> (c s) j", j=512), ob[:]
                )

    nc.compile()
    return nc


def make_in_maps(inputs, pack):
    import ml_dtypes
    x = np.ascontiguousarray(np.asarray(inputs["x"], np.float32))
    xb = x.astype(ml_dtypes.bfloat16)
    curves = np.ascontiguousarray(np.asarray(inputs["curves"], np.float32))
    in_maps = []
    for i in range(NCORES):
        in_maps.append({
            "curves": curves,
            "cpack": pack,
            "x_sh": np.ascontiguousarray(x[:, :, i * NS: (i + 1) * NS]),
            "x_bf": np.ascontiguousarray(xb[:, :, i * NS: (i + 1) * NS]),
        })
    return in_maps


def kernel(**inputs):
    from concourse.bass_utils import run_bass_kernel_spmd

    pack, layout, bbar = _host_consts(inputs)

    key = ("v8", bbar, pack.shape, tuple(sorted(layout.items())))
    if key not in _cache:
        _cache[key] = _build(pack.shape, layout, bbar)
    nc = _cache[key]

    in_maps = make_in_maps(inputs, pack)
    res = run_bass_kernel_spmd(nc, in_maps, core_ids=list(range(NCORES)))
    out = np.empty((B, C, N), np.float32)
    for i in range(NCORES):
        out[:, :, i * NS: (i + 1) * NS] = res.results[i]["out"]
    return out

